# revision 27
# baseline (speedup 1.0000x reference)
"""Trainium2 Bass kernel for nn_BatchCrop (scatter_memory problem).

reference computation:
  patch_norm = sum_m waves[m]^2                 -> [B, 128, 128]
  object_norm = scatter-add(patch_norm @ pos)   -> [2048, 2048]
  patches     = gather(obj @ pos)               -> [B, 1, 128, 128]

Strategy (8 cores, SPMD single NEFF):
  * Host sorts batch by row position r0 and shards 128 patches/core, so each
    core's scatter targets a narrow row band -> per-core DRAM partials, host
    sums the small boundary overlaps (no on-device all-reduce needed).
  * waves streamed in h-major slabs (patch row j on partition j), ACT
    squares, DVE pairwise adds -> patch_norm SBUF buffer [128, B*128].
  * All data-dependent addressing uses indirect DMA with one int32 element
    offset per partition (the only dynamic addressing mechanism this
    runtime supports; register-offset DMAs crash the exec unit and
    multi-index offset tables are ignored by the DGE ucode).
  * patches: per-patch indirect gather obj->SBUF staging, then a static
    HWDGE DMA staging->patches_out.
  * object_norm: per-patch indirect scatter-add (compute_op=add) into one
    of NT=4 row-quartile DRAM partials. Tile's conservative whole-tensor
    tracking serializes same-tensor scatters via DMA-completion sems
    (correct for any overlap); round-robin issue over the 4 tensors keeps
    4 chains in flight so the Q7 descriptor generator stays busy.
"""

import sys

for _p in ("/opt/trn_rl_repo",):
    if _p not in sys.path:
        sys.path.insert(0, _p)

import numpy as np

OBJ_H = OBJ_W = 2048
PH = PW = 128
N_MODES = 4
N_CORES = 8
BC = 128   # patches per core
BS = 8     # patches per compute slab
N_SLAB = BC // BS
NT = 4     # number of partial accumulation tensors (row quartiles)
QG = BC // NT  # patches per quartile group


# --------------------------------------------------------------------------
# host-side prep
# --------------------------------------------------------------------------

def _prep(pos):
    """Shard + build per-core offset tables. Returns (cores, Ht)."""
    r0 = pos[:, 0].astype(np.int64)
    order = np.argsort(r0, kind="stable")
    shards = [order[k * BC:(k + 1) * BC] for k in range(N_CORES)]

    # slot b holds the (b // NT)-th patch of r0-quartile (b % NT), so that
    # consecutive slots cycle through the NT scatter chains and each compute
    # slab feeds all chains evenly
    slot_perm = np.array([(b % NT) * QG + b // NT for b in range(BC)])

    cores = []
    Ht = 0
    for k in range(N_CORES):
        idx = shards[k][slot_perm]
        rk = pos[idx, 0].astype(np.int64)
        ck = pos[idx, 1].astype(np.int64)
        bases = np.empty(NT, np.int64)
        for t in range(NT):
            grp = rk[t::NT]
            bases[t] = grp.min()
            Ht = max(Ht, int(grp.max()) + PH - bases[t])
        cores.append(dict(idx=idx, r0=rk, c0=ck, bases=bases))
    Ht = (int(Ht) + 127) & ~127

    p = np.arange(PH, dtype=np.int64)[:, None]
    for cd in cores:
        rk, ck, bases = cd["r0"], cd["c0"], cd["bases"]
        rrel = rk - np.tile(bases, QG)
        assert rrel.min() >= 0 and rrel.max() + PH <= Ht
        # [p, b] element offsets
        cd["gtab"] = ((rk[None, :] + p) * OBJ_W + ck[None, :]).astype(np.int32)
        cd["stab"] = ((rrel[None, :] + p) * OBJ_W + ck[None, :]).astype(np.int32)
    return cores, Ht


# --------------------------------------------------------------------------
# device kernel
# --------------------------------------------------------------------------

def _build(Ht, cost_stub=False, reps=1, delay=2):
    from concourse import bacc, bass, mybir
    import concourse.tile as tile

    f32 = mybir.dt.float32
    i32 = mybir.dt.int32

    nc = bacc.Bacc(None, name="batchcrop")
    obj = nc.dram_tensor("obj", [OBJ_H * OBJ_W, 1], f32, kind="ExternalInput")
    waves = nc.dram_tensor("waves", [N_MODES, BC, PH, PW], f32,
                           kind="ExternalInput")
    gtab = nc.dram_tensor("gtab", [PH, BC], i32, kind="ExternalInput")
    stab = nc.dram_tensor("stab", [PH, BC], i32, kind="ExternalInput")
    patches = nc.dram_tensor("patches", [BC, PH, PW], f32,
                             kind="ExternalOutput")
    partials = [
        nc.dram_tensor(f"partial_{t}", [Ht * OBJ_W, 1], f32,
                       kind="ExternalOutput")
        for t in range(NT)
    ]
    if cost_stub:
        # planning-only twin: the cost model bills an indirect DMA by its
        # declared AP extent, so swap the whole-tensor indirect sides for
        # transfer-sized stand-ins (keeps per-chain dep structure intact)
        obj_i = nc.dram_tensor("obj_stub", [PH, PW], f32,
                               kind="ExternalInput")
        partials_i = [
            nc.dram_tensor(f"pstub_{t}", [PH, PW], f32,
                           kind="ExternalOutput")
            for t in range(NT)
        ]
    else:
        obj_i = obj
        partials_i = partials

    with tile.TileContext(nc) as tc:
        with (
            tc.tile_pool(name="const", bufs=1) as cpool,
            tc.tile_pool(name="wv", bufs=2) as wvp,
            tc.tile_pool(name="sq", bufs=2) as sqp,
            tc.tile_pool(name="tt", bufs=2) as ttp,
            tc.tile_pool(name="pn", bufs=1) as pnp,
            tc.tile_pool(name="stag", bufs=6) as stp,
        ):
            gtab_t = cpool.tile([PH, BC], i32)
            nc.sync.dma_start(gtab_t[:], gtab[:])
            stab_t = cpool.tile([PH, BC], i32)
            nc.sync.dma_start(stab_t[:], stab[:])

            # zero the partials
            zero_t = cpool.tile([128, 2 * OBJ_W], f32)
            nc.vector.memset(zero_t[:], 0)
            zn = Ht // 256
            for p in partials:
                for z in range(zn):
                    nc.sync.dma_start(
                        p[z * 256 * OBJ_W:(z + 1) * 256 * OBJ_W], zero_t[:])

            # main loop: per slab of BS patches, interleave on Pool:
            # [BS gathers][BS scatter-adds of the previous-ish slab's pn].
            # Slot b's scatter targets chain b % NT, so each group of BS=8
            # cycles through all NT chains twice (completion latency hides
            # behind the other chains + the gathers).
            pn_buf = pnp.tile([PH, BC * PW], f32)
            for s in range(N_SLAB * reps):
                s = s % N_SLAB
                lo = s * BS
                # gathers for this slab's slots
                for b in range(lo, lo + BS):
                    stag = stp.tile([PH, PW], f32, tag="stag")
                    nc.gpsimd.indirect_dma_start(
                        out=stag[:], out_offset=None, in_=obj_i[:],
                        in_offset=bass.IndirectOffsetOnAxis(
                            ap=gtab_t[:, b:b + 1], axis=0))
                    eng = nc.sync if b % 2 == 0 else nc.scalar
                    eng.dma_start(patches[b], stag[:])
                # slab load + patch_norm compute
                wv = wvp.tile([PH, N_MODES * BS * PW], f32, tag="wv")
                for m in range(N_MODES):
                    nc.sync.dma_start(
                        wv[:, m * BS * PW:(m + 1) * BS * PW],
                        waves[m, lo:lo + BS, :, :].transpose([1, 0, 2]),
                    )
                sq = sqp.tile([PH, N_MODES * BS * PW], f32, tag="sq")
                nc.scalar.activation(sq[:], wv[:],
                                     mybir.ActivationFunctionType.Square)
                half = 2 * BS * PW
                t = ttp.tile([PH, half], f32, tag="t")
                nc.vector.tensor_add(t[:], sq[:, :half], sq[:, half:])
                nc.vector.tensor_add(
                    pn_buf[:, lo * PW:(lo + BS) * PW],
                    t[:, :BS * PW], t[:, BS * PW:])
                # scatter-adds, delayed by `delay` slabs so Pool never
                # stalls waiting for the current slab's patch_norm
                if s >= delay:
                    dlo = (s - delay) * BS
                    for b in range(dlo, dlo + BS):
                        nc.gpsimd.indirect_dma_start(
                            out=partials_i[b % NT][:],
                            out_offset=bass.IndirectOffsetOnAxis(
                                ap=stab_t[:, b:b + 1], axis=0),
                            in_=pn_buf[:, b * PW:(b + 1) * PW],
                            in_offset=None,
                            compute_op=mybir.AluOpType.add)
            for s in range(max(N_SLAB - delay, 0), N_SLAB):
                for b in range(s * BS, s * BS + BS):
                    nc.gpsimd.indirect_dma_start(
                        out=partials_i[b % NT][:],
                        out_offset=bass.IndirectOffsetOnAxis(
                            ap=stab_t[:, b:b + 1], axis=0),
                        in_=pn_buf[:, b * PW:(b + 1) * PW],
                        in_offset=None,
                        compute_op=mybir.AluOpType.add)
    return nc


# --------------------------------------------------------------------------
# entry point
# --------------------------------------------------------------------------

def kernel(obj, waves, pos, _trace=False):
    from concourse.bass_utils import run_bass_kernel_spmd

    obj = np.asarray(obj, dtype=np.float32)
    waves = np.asarray(waves, dtype=np.float32)
    pos = np.asarray(pos, dtype=np.int32)

    cores, Ht = _prep(pos)
    nc = _build(Ht)
    nc.finalize()

    obj_flat = np.ascontiguousarray(obj.reshape(-1, 1))
    in_maps = []
    for cd in cores:
        in_maps.append({
            "obj": obj_flat,
            "waves": np.ascontiguousarray(waves[:, cd["idx"]]),
            "gtab": cd["gtab"],
            "stab": cd["stab"],
        })

    res = run_bass_kernel_spmd(nc, in_maps, core_ids=list(range(N_CORES)),
                               trace=_trace)

    patches_full = np.empty((pos.shape[0], 1, PH, PW), dtype=np.float32)
    object_norm = np.zeros((OBJ_H, OBJ_W), dtype=np.float32)
    for k, cd in enumerate(cores):
        out = res.results[k]
        patches_full[cd["idx"], 0] = out["patches"]
        for t in range(NT):
            base = int(cd["bases"][t])
            rows = min(Ht, OBJ_H - base)
            object_norm[base:base + rows] += out[f"partial_{t}"].reshape(
                Ht, OBJ_W)[:rows]

    kernel.last_results = res
    kernel.last_Ht = Ht
    return patches_full, object_norm
